# revision 2
# baseline (speedup 1.0000x reference)
"""Trainium2 Bass kernel for nn_MinimalLoss (YOLO-style detection loss).

Strategy (data-parallel over 8 NeuronCores, 4 batches each):
  The only parts of `predictions` [B, HW, 85] that matter are:
    * column 4 (conf logit) of every cell  -> sum of -ln(1-sigmoid(x))
    * the <=200 rows per core addressed by targets -> gathered via
      indirect DMA; xy/wh/cls/conf-correction terms computed on-chip.
  Duplicate-cell targets are deduplicated on-chip (obj_mask semantics of
  the reference scatter-max) with a transpose/is_equal first-occurrence
  matrix. Per-core partial sums (6 scalars) are combined on host.
"""
import os

import numpy as np

import concourse.bass as bass
import concourse.mybir as mybir
import concourse.tile as tile
from concourse.bass import IndirectOffsetOnAxis
from concourse.masks import make_identity

F32 = mybir.dt.float32
I32 = mybir.dt.int32
AF = mybir.ActivationFunctionType
ALU = mybir.AluOpType
AX = mybir.AxisListType

B, HWC, C, T = 32, 25600, 80, 50          # full problem
H = W = 160
NCORES = 8
BL = B // NCORES                          # 4 batches per core
ROWS = BL * HWC                           # 102400 prediction rows per core
NT = BL * T                               # 200 targets per core
HALF = NT // 2                            # 100 targets per half (2 batches)
MAGIC = float(np.float32(2 ** 23))

# conf-channel pass configuration
CONF_VARIANT = os.environ.get("CONF_VARIANT", "strided")  # strided | bulk
NCH = int(os.environ.get("CONF_NCH", "8"))                # strided: chunks of 800/NCH cols
BULK_R = 100                                              # bulk: rows/partition/chunk
CONF_DMA = os.environ.get("CONF_DMA", "sync")           # gpsimd | sync


def _conf_pass_strided(nc, cp, sb, pred_ap, acc):
    """acc[:, k] = per-partition sums of ln(1-sigmoid(conf))."""
    conf = pred_ap[:, 4:5].rearrange("(p j) o -> p (j o)", p=128)  # [128, 800]
    cw = 800 // NCH
    for k in range(NCH):
        if CONF_DMA == "dual":
            dma_eng = nc.sync if k % 2 == 0 else nc.scalar
        else:
            dma_eng = nc.gpsimd if CONF_DMA == "gpsimd" else nc.sync
        # dedicated all-live pool: a slot is never reused, so each DMA needs
        # <=1 sync wait (DIRECT2D codegen limit)
        tl = cp.tile([128, cw], F32, tag="conf_in")
        dma_eng.dma_start(out=tl[:], in_=conf[:, k * cw:(k + 1) * cw])
        om = cp.tile([128, cw], F32, tag="conf_om")
        nc.scalar.activation(out=om[:], in_=tl[:], func=AF.Sigmoid)
        nc.vector.tensor_scalar(out=om[:], in0=om[:], scalar1=1.0, scalar2=-1.0,
                                op0=ALU.subtract, op1=ALU.mult)
        ln = cp.tile([128, cw], F32, tag="conf_ln")
        nc.scalar.activation(out=ln[:], in_=om[:], func=AF.Ln, accum_out=acc[:, k:k + 1])


def _conf_pass_bulk(nc, sb, pred_ap, acc):
    """Bulk-load full rows; extract conf with a strided on-chip read."""
    flat = pred_ap.rearrange("r c -> (r c)").rearrange("(p j) -> p j", p=128)  # [128, 800*85]
    nch = 800 // BULK_R
    for k in range(nch):
        tl = sb.tile([128, BULK_R * 85], F32, tag="bulk_in")
        nc.sync.dma_start(out=tl[:], in_=flat[:, k * BULK_R * 85:(k + 1) * BULK_R * 85])
        cv = tl[:].rearrange("p (j c) -> p j c", c=85)[:, :, 4:5].rearrange("p j o -> p (j o)")
        om = sb.tile([128, BULK_R], F32, tag="bulk_om")
        nc.scalar.activation(out=om[:], in_=cv, func=AF.Sigmoid)
        nc.vector.tensor_scalar(out=om[:], in0=om[:], scalar1=1.0, scalar2=-1.0,
                                op0=ALU.subtract, op1=ALU.mult)
        ln = sb.tile([128, BULK_R], F32, tag="bulk_ln")
        nc.scalar.activation(out=ln[:], in_=om[:], func=AF.Ln, accum_out=acc[:, k:k + 1])


def _floor(nc, sb, dst, src, n):
    """dst = floor(src) for 0 <= src < 2^22, exact (round-to-nearest fixup)."""
    r = sb.tile([n, 1], F32, tag="fl_r")
    adj = sb.tile([n, 1], F32, tag="fl_a")
    nc.vector.tensor_scalar_add(r[:], src, MAGIC)
    nc.vector.tensor_scalar_add(r[:], r[:], -MAGIC)
    nc.vector.tensor_tensor(out=adj[:], in0=r[:], in1=src, op=ALU.is_gt)
    nc.vector.tensor_tensor(out=dst, in0=r[:], in1=adj[:], op=ALU.subtract)


def _split_multi_waits(nc):
    """Walrus codegen accepts at most ONE sync wait per instruction; hoist
    extras onto standalone EventSemaphore (wait) ops on the same engine."""
    n = 0
    for func in nc.m.functions:
        for block in func.blocks:
            out = []
            for inst in block.instructions:
                si = inst.sync_info
                if si is not None and si.on_wait and len(si.on_wait) > 1:
                    waits = list(si.on_wait)
                    for w in waits[:-1]:
                        n += 1
                        nop = mybir.InstEventSemaphore(
                            name=f"{inst.name}_sw{n}", engine=inst.engine,
                            ins=[], outs=[])
                        nop.sync_info = mybir.SyncInfo(on_wait=[w], on_update=[])
                        out.append(nop)
                    inst.sync_info = mybir.SyncInfo(on_wait=[waits[-1]],
                                                    on_update=list(si.on_update))
                out.append(inst)
            if n:
                block.instructions[:] = out
    return n


def build_nc(split=True):
    nc = bass.Bass("TRN2", target_bir_lowering=False, debug=False)
    pred_d = nc.dram_tensor("predictions", [ROWS, 85], F32, kind="ExternalInput")
    tgt_d = nc.dram_tensor("targets", [NT, 5], F32, kind="ExternalInput")
    out_d = nc.dram_tensor("out", [8, 1], F32, kind="ExternalOutput")

    pred_ap = pred_d.ap()
    n_conf_cols = NCH if CONF_VARIANT == "strided" else 800 // BULK_R

    with tile.TileContext(nc) as tc:
        with tc.tile_pool(name="persist", bufs=1) as pp, \
             tc.tile_pool(name="conf", bufs=NCH) as cp, \
             tc.tile_pool(name="sb", bufs=2) as sb, \
             tc.tile_pool(name="ps", bufs=1, space="PSUM") as ps:

            acc = pp.tile([128, n_conf_cols], F32)

            # constants (route matmul operands through DVE so each matmul
            # needs at most ONE sync wait — the S3_LW slot limit)
            ident_g = pp.tile([128, 128], F32)
            make_identity(nc, ident_g[:])
            ident = pp.tile([128, 128], F32)
            nc.vector.tensor_copy(out=ident[:], in_=ident_g[:])
            ones = pp.tile([128, 1], F32)
            nc.vector.memset(ones[:], 1.0)
            iotac = pp.tile([128, C], I32)
            nc.gpsimd.iota(iotac[:], pattern=[[1, C]], base=0, channel_multiplier=0)
            iotaf = pp.tile([128, C], F32)
            nc.vector.tensor_copy(out=iotaf[:], in_=iotac[:])
            iotap = pp.tile([128, 1], I32)
            nc.gpsimd.iota(iotap[:], pattern=[[1, 1]], base=0, channel_multiplier=1)
            pf128 = pp.tile([128, 1], F32)
            nc.vector.tensor_copy(out=pf128[:], in_=iotap[:])
            iotar = pp.tile([128, 128], I32)
            nc.gpsimd.iota(iotar[:], pattern=[[1, 128]], base=0, channel_multiplier=0)
            iotarf = pp.tile([128, 128], F32)
            nc.vector.tensor_copy(out=iotarf[:], in_=iotar[:])
            tri = pp.tile([128, 128], F32)  # tri[p, f] = 1.0 iff f < p
            nc.vector.tensor_tensor(out=tri[:], in0=pf128[:].to_broadcast([128, 128]),
                                    in1=iotarf[:], op=ALU.is_gt)

            # ---- conf channel: sum ln(1-sigmoid(x)) over all cells
            if CONF_VARIANT == "strided":
                _conf_pass_strided(nc, cp, sb, pred_ap, acc)
            else:
                _conf_pass_bulk(nc, sb, pred_ap, acc)

            # ---- per-target phase: two halves of 100 targets (2 whole batches each)
            P = HALF
            stats_ps = ps.tile([5, 1], F32, space="PSUM")
            for q in range(2):
                tt = sb.tile([P, 5], F32, tag="tt")
                nc.sync.dma_start(out=tt[:], in_=tgt_d.ap()[q * P:(q + 1) * P, :])

                xW = sb.tile([P, 1], F32, tag="xW")
                yH = sb.tile([P, 1], F32, tag="yH")
                nc.vector.tensor_scalar_mul(xW[:], tt[:, 1:2], float(W))
                nc.vector.tensor_scalar_mul(yH[:], tt[:, 2:3], float(H))
                gx = sb.tile([P, 1], F32, tag="gx")
                gy = sb.tile([P, 1], F32, tag="gy")
                _floor(nc, sb, gx[:], xW[:], P)
                _floor(nc, sb, gy[:], yH[:], P)

                # validity
                vf = sb.tile([P, 1], F32, tag="vf")
                tmp = sb.tile([P, 1], F32, tag="tmp")
                nc.vector.tensor_scalar(out=vf[:], in0=gx[:], scalar1=0.0, scalar2=None, op0=ALU.is_ge)
                nc.vector.tensor_scalar(out=tmp[:], in0=gx[:], scalar1=float(W), scalar2=None, op0=ALU.is_lt)
                nc.vector.tensor_tensor(out=vf[:], in0=vf[:], in1=tmp[:], op=ALU.mult)
                nc.vector.tensor_scalar(out=tmp[:], in0=gy[:], scalar1=0.0, scalar2=None, op0=ALU.is_ge)
                nc.vector.tensor_tensor(out=vf[:], in0=vf[:], in1=tmp[:], op=ALU.mult)
                nc.vector.tensor_scalar(out=tmp[:], in0=gy[:], scalar1=float(H), scalar2=None, op0=ALU.is_lt)
                nc.vector.tensor_tensor(out=vf[:], in0=vf[:], in1=tmp[:], op=ALU.mult)

                # cell + per-core row index
                gxi = sb.tile([P, 1], F32, tag="gxi")
                gyi = sb.tile([P, 1], F32, tag="gyi")
                nc.vector.tensor_scalar(out=gxi[:], in0=gx[:], scalar1=0.0, scalar2=float(W - 1),
                                        op0=ALU.max, op1=ALU.min)
                nc.vector.tensor_scalar(out=gyi[:], in0=gy[:], scalar1=0.0, scalar2=float(H - 1),
                                        op0=ALU.max, op1=ALU.min)
                cell = sb.tile([P, 1], F32, tag="cell")
                nc.vector.tensor_scalar_mul(cell[:], gyi[:], float(W))
                nc.vector.tensor_tensor(out=cell[:], in0=cell[:], in1=gxi[:], op=ALU.add)

                rowf = sb.tile([P, 1], F32, tag="rowf")
                # batch offset: (2q + (t>=50)) * HWC
                nc.vector.tensor_scalar(out=rowf[:], in0=pf128[:P, :], scalar1=float(T), scalar2=None,
                                        op0=ALU.is_ge)
                nc.vector.tensor_scalar(out=rowf[:], in0=rowf[:], scalar1=float(HWC),
                                        scalar2=float(2 * q * HWC), op0=ALU.mult, op1=ALU.add)
                nc.vector.tensor_tensor(out=rowf[:], in0=rowf[:], in1=cell[:], op=ALU.add)
                idx = sb.tile([P, 1], I32, tag="idx")
                nc.vector.tensor_copy(out=idx[:], in_=rowf[:])

                # dedup key: valid -> rowf ; invalid -> unique negative
                negk = sb.tile([P, 1], F32, tag="negk")
                nc.vector.tensor_scalar(out=negk[:], in0=pf128[:P, :], scalar1=-1.0,
                                        scalar2=-(1.0 + 100.0 * q), op0=ALU.mult, op1=ALU.add)
                key = sb.tile([P, 1], F32, tag="key")
                nc.vector.tensor_tensor(out=key[:], in0=rowf[:], in1=negk[:], op=ALU.subtract)
                nc.vector.tensor_tensor(out=key[:], in0=key[:], in1=vf[:], op=ALU.mult)
                nc.vector.tensor_tensor(out=key[:], in0=key[:], in1=negk[:], op=ALU.add)

                # gather prediction rows
                rows = sb.tile([P, 85], F32, tag="rows")
                nc.gpsimd.indirect_dma_start(
                    out=rows[:], out_offset=None, in_=pred_ap[:, :],
                    in_offset=IndirectOffsetOnAxis(ap=idx[:, :1], axis=0))

                # sigmoid/ln terms over the whole row
                sg = sb.tile([P, 85], F32, tag="sg")
                nc.scalar.activation(out=sg[:], in_=rows[:], func=AF.Sigmoid)
                lnp = sb.tile([P, 85], F32, tag="lnp")
                nc.scalar.activation(out=lnp[:], in_=sg[:], func=AF.Ln)
                nc.vector.tensor_scalar_max(lnp[:], lnp[:], -100.0)
                om = sb.tile([P, 85], F32, tag="om")
                nc.vector.tensor_scalar(out=om[:], in0=sg[:], scalar1=1.0, scalar2=-1.0,
                                        op0=ALU.subtract, op1=ALU.mult)
                lnn = sb.tile([P, 85], F32, tag="lnn")
                nc.scalar.activation(out=lnn[:], in_=om[:], func=AF.Ln)
                nc.vector.tensor_scalar_max(lnn[:], lnn[:], -100.0)

                # per_cls = -(1/C) * sum_c [ onehot*lnp + (1-onehot)*lnn ]
                oh = sb.tile([P, C], F32, tag="oh")
                nc.vector.tensor_tensor(out=oh[:], in0=iotaf[:P, :],
                                        in1=tt[:, 0:1].to_broadcast([P, C]), op=ALU.is_equal)
                dlt = sb.tile([P, C], F32, tag="dlt")
                nc.vector.tensor_tensor(out=dlt[:], in0=lnp[:, 5:85], in1=lnn[:, 5:85], op=ALU.subtract)
                nc.vector.tensor_tensor(out=dlt[:], in0=dlt[:], in1=oh[:], op=ALU.mult)
                nc.vector.tensor_tensor(out=dlt[:], in0=dlt[:], in1=lnn[:, 5:85], op=ALU.add)
                pcls = sb.tile([P, 1], F32, tag="pcls")
                nc.vector.reduce_sum(out=pcls[:], in_=dlt[:], axis=AX.X)
                nc.vector.tensor_scalar_mul(pcls[:], pcls[:], -1.0 / C)

                # conf correction term: ct = lnn[4] - lnp[4]  ( = term_pos - term_neg )
                ct = sb.tile([P, 1], F32, tag="ct")
                nc.vector.tensor_tensor(out=ct[:], in0=lnn[:, 4:5], in1=lnp[:, 4:5], op=ALU.subtract)

                # per_xy / per_wh
                txy = sb.tile([P, 2], F32, tag="txy")
                nc.vector.tensor_tensor(out=txy[:, 0:1], in0=xW[:], in1=gx[:], op=ALU.subtract)
                nc.vector.tensor_tensor(out=txy[:, 1:2], in0=yH[:], in1=gy[:], op=ALU.subtract)
                dxy = sb.tile([P, 2], F32, tag="dxy")
                nc.vector.tensor_tensor(out=dxy[:], in0=sg[:, 0:2], in1=txy[:], op=ALU.subtract)
                nc.vector.tensor_tensor(out=dxy[:], in0=dxy[:], in1=dxy[:], op=ALU.mult)
                pxy = sb.tile([P, 1], F32, tag="pxy")
                nc.vector.reduce_sum(out=pxy[:], in_=dxy[:], axis=AX.X)
                nc.vector.tensor_scalar_mul(pxy[:], pxy[:], 0.5)

                pwh_t = sb.tile([P, 2], F32, tag="pwh")
                nc.scalar.activation(out=pwh_t[:], in_=rows[:, 2:4], func=AF.Exp)
                twh = sb.tile([P, 2], F32, tag="twh")
                nc.vector.tensor_scalar_mul(twh[:, 0:1], tt[:, 3:4], float(W))
                nc.vector.tensor_scalar_mul(twh[:, 1:2], tt[:, 4:5], float(H))
                dwh = sb.tile([P, 2], F32, tag="dwh")
                nc.vector.tensor_tensor(out=dwh[:], in0=pwh_t[:], in1=twh[:], op=ALU.subtract)
                nc.vector.tensor_tensor(out=dwh[:], in0=dwh[:], in1=dwh[:], op=ALU.mult)
                pwh = sb.tile([P, 1], F32, tag="pwh1")
                nc.vector.reduce_sum(out=pwh[:], in_=dwh[:], axis=AX.X)
                nc.vector.tensor_scalar_mul(pwh[:], pwh[:], 0.5)

                # dedup: first-occurrence weight w
                keyT_ps = ps.tile([P, P], F32, space="PSUM", tag="keyT_ps")
                nc.tensor.transpose(out=keyT_ps[:], in_=key[:].to_broadcast([P, P]),
                                    identity=ident[:P, :P])
                keyT = sb.tile([P, P], F32, tag="keyT")
                nc.vector.tensor_copy(out=keyT[:], in_=keyT_ps[:])
                eq = sb.tile([P, P], F32, tag="eq")
                nc.vector.tensor_tensor(out=eq[:], in0=key[:].to_broadcast([P, P]),
                                        in1=keyT[:], op=ALU.is_equal)
                nc.vector.tensor_tensor(out=eq[:], in0=eq[:], in1=tri[:P, :P], op=ALU.mult)
                dup = sb.tile([P, 1], F32, tag="dup")
                nc.vector.reduce_max(out=dup[:], in_=eq[:], axis=AX.X)
                wfo = sb.tile([P, 1], F32, tag="wfo")
                nc.vector.tensor_scalar(out=wfo[:], in0=dup[:], scalar1=-1.0, scalar2=1.0,
                                        op0=ALU.mult, op1=ALU.add)
                nc.vector.tensor_tensor(out=wfo[:], in0=wfo[:], in1=vf[:], op=ALU.mult)

                # stats columns: vf*pxy, vf*pwh, vf*pcls, vf, w*ct
                stats = sb.tile([P, 5], F32, tag="stats")
                nc.vector.tensor_tensor(out=stats[:, 0:1], in0=pxy[:], in1=vf[:], op=ALU.mult)
                nc.vector.tensor_tensor(out=stats[:, 1:2], in0=pwh[:], in1=vf[:], op=ALU.mult)
                nc.vector.tensor_tensor(out=stats[:, 2:3], in0=pcls[:], in1=vf[:], op=ALU.mult)
                nc.vector.tensor_copy(out=stats[:, 3:4], in_=vf[:])
                nc.vector.tensor_tensor(out=stats[:, 4:5], in0=ct[:], in1=wfo[:], op=ALU.mult)

                nc.tensor.matmul(out=stats_ps[:], lhsT=stats[:], rhs=ones[:P, :],
                                 start=(q == 0), stop=(q == 1))

            # ---- final reductions
            racc = pp.tile([128, 1], F32)
            nc.vector.reduce_sum(out=racc[:], in_=acc[:], axis=AX.X)
            conf_ps = ps.tile([1, 1], F32, space="PSUM")
            nc.tensor.matmul(out=conf_ps[:], lhsT=ones[:], rhs=racc[:], start=True, stop=True)

            so = pp.tile([5, 1], F32)
            nc.vector.tensor_copy(out=so[:], in_=stats_ps[:])
            co = pp.tile([1, 1], F32)
            nc.vector.tensor_copy(out=co[:], in_=conf_ps[:])
            nc.gpsimd.dma_start(out=out_d.ap()[0:5, :], in_=so[:])
            nc.gpsimd.dma_start(out=out_d.ap()[5:6, :], in_=co[:])
    if split:
        _split_multi_waits(nc)
    return nc


_NC_CACHE = None


def _get_nc():
    global _NC_CACHE
    if _NC_CACHE is None:
        _NC_CACHE = build_nc()
    return _NC_CACHE


def make_in_maps(predictions, targets):
    preds = np.ascontiguousarray(np.asarray(predictions, dtype=np.float32)).reshape(NCORES, ROWS, 85)
    tgts = np.ascontiguousarray(np.asarray(targets, dtype=np.float32)).reshape(NCORES, NT, 5)
    return [{"predictions": preds[c], "targets": tgts[c]} for c in range(NCORES)]


def combine_partials(parts):
    """parts: list of 8 arrays [8,1] -> (total, loss_xy, loss_wh, loss_conf, loss_cls)"""
    s = np.sum([p.reshape(-1) for p in parts], axis=0, dtype=np.float64)
    xy, wh, cls_, nt, corr, lnsum = [np.float32(v) for v in s[:6]]
    denom = np.float32(max(float(nt), 1.0))
    loss_xy = np.float32(xy / denom)
    loss_wh = np.float32(wh / denom)
    loss_cls = np.float32(cls_ / denom)
    loss_conf = np.float32((-lnsum + corr) / np.float32(B * HWC))
    total = np.float32(5.0 * loss_xy + 5.0 * loss_wh + loss_conf + loss_cls)
    return total, loss_xy, loss_wh, loss_conf, loss_cls


def kernel(predictions, targets, H=None, W=None):
    from concourse.bass_utils import run_bass_kernel_spmd

    nc = _get_nc()
    in_maps = make_in_maps(predictions, targets)
    res = run_bass_kernel_spmd(nc, in_maps, core_ids=list(range(NCORES)))
    parts = [res.results[c]["out"] for c in range(NCORES)]
    return combine_partials(parts)



# revision 3
# speedup vs baseline: 1.0405x; 1.0405x over previous
"""Trainium2 Bass kernel for nn_MinimalLoss (YOLO-style detection loss).

Strategy (data-parallel over 8 NeuronCores, 4 batches each):
  The only parts of `predictions` [B, HW, 85] that matter are:
    * column 4 (conf logit) of every cell  -> sum of -ln(1-sigmoid(x))
    * the <=200 rows per core addressed by targets -> gathered via
      indirect DMA; xy/wh/cls/conf-correction terms computed on-chip.
  Duplicate-cell targets are deduplicated on-chip (obj_mask semantics of
  the reference scatter-max) with a transpose/is_equal first-occurrence
  matrix. Per-core partial sums (6 scalars) are combined on host.
"""
import os

import numpy as np

import concourse.bass as bass
import concourse.mybir as mybir
import concourse.tile as tile
from concourse.bass import IndirectOffsetOnAxis
from concourse.masks import make_identity

F32 = mybir.dt.float32
I32 = mybir.dt.int32
AF = mybir.ActivationFunctionType
ALU = mybir.AluOpType
AX = mybir.AxisListType

B, HWC, C, T = 32, 25600, 80, 50          # full problem
H = W = 160
NCORES = 8
BL = B // NCORES                          # 4 batches per core
ROWS = BL * HWC                           # 102400 prediction rows per core
NT = BL * T                               # 200 targets per core
HALF = NT // 2                            # 100 targets per half (2 batches)
MAGIC = float(np.float32(2 ** 23))

# conf-channel pass configuration
CONF_VARIANT = os.environ.get("CONF_VARIANT", "strided")  # strided | bulk
NCH = int(os.environ.get("CONF_NCH", "8"))                # strided: chunks of 800/NCH cols
BULK_R = 100                                              # bulk: rows/partition/chunk
CONF_DMA = os.environ.get("CONF_DMA", "sync")           # gpsimd | sync


def _conf_pass_strided(nc, cp, sb, pred_ap, acc):
    """acc[:, k] = per-partition sums of ln(1-sigmoid(conf))."""
    conf = pred_ap[:, 4:5].rearrange("(p j) o -> p (j o)", p=128)  # [128, 800]
    cw = 800 // NCH
    for k in range(NCH):
        if CONF_DMA == "dual":
            dma_eng = nc.sync if k % 2 == 0 else nc.scalar
        else:
            dma_eng = nc.gpsimd if CONF_DMA == "gpsimd" else nc.sync
        # dedicated all-live pool: a slot is never reused, so each DMA needs
        # <=1 sync wait (DIRECT2D codegen limit)
        tl = cp.tile([128, cw], F32, tag="conf_in")
        dma_eng.dma_start(out=tl[:], in_=conf[:, k * cw:(k + 1) * cw],
                          single_packet=os.environ.get("CONF_SP", "0") == "1")
        om = cp.tile([128, cw], F32, tag="conf_om")
        nc.scalar.activation(out=om[:], in_=tl[:], func=AF.Sigmoid)
        nc.vector.tensor_scalar(out=om[:], in0=om[:], scalar1=1.0, scalar2=-1.0,
                                op0=ALU.subtract, op1=ALU.mult)
        ln = cp.tile([128, cw], F32, tag="conf_ln")
        nc.scalar.activation(out=ln[:], in_=om[:], func=AF.Ln, accum_out=acc[:, k:k + 1])


def _conf_pass_bulk(nc, sb, pred_ap, acc):
    """Bulk-load full rows; extract conf with a strided on-chip read."""
    flat = pred_ap.rearrange("r c -> (r c)").rearrange("(p j) -> p j", p=128)  # [128, 800*85]
    nch = 800 // BULK_R
    for k in range(nch):
        tl = sb.tile([128, BULK_R * 85], F32, tag="bulk_in")
        nc.sync.dma_start(out=tl[:], in_=flat[:, k * BULK_R * 85:(k + 1) * BULK_R * 85])
        cv = tl[:].rearrange("p (j c) -> p j c", c=85)[:, :, 4:5].rearrange("p j o -> p (j o)")
        om = sb.tile([128, BULK_R], F32, tag="bulk_om")
        nc.scalar.activation(out=om[:], in_=cv, func=AF.Sigmoid)
        nc.vector.tensor_scalar(out=om[:], in0=om[:], scalar1=1.0, scalar2=-1.0,
                                op0=ALU.subtract, op1=ALU.mult)
        ln = sb.tile([128, BULK_R], F32, tag="bulk_ln")
        nc.scalar.activation(out=ln[:], in_=om[:], func=AF.Ln, accum_out=acc[:, k:k + 1])


def _floor(nc, sb, dst, src, n):
    """dst = floor(src) for 0 <= src < 2^22, exact (round-to-nearest fixup)."""
    r = sb.tile([n, 1], F32, tag="fl_r")
    adj = sb.tile([n, 1], F32, tag="fl_a")
    nc.vector.tensor_scalar_add(r[:], src, MAGIC)
    nc.vector.tensor_scalar_add(r[:], r[:], -MAGIC)
    nc.vector.tensor_tensor(out=adj[:], in0=r[:], in1=src, op=ALU.is_gt)
    nc.vector.tensor_tensor(out=dst, in0=r[:], in1=adj[:], op=ALU.subtract)


def _split_multi_waits(nc):
    """Walrus codegen accepts at most ONE sync wait per instruction; hoist
    extras onto standalone EventSemaphore (wait) ops on the same engine."""
    n = 0
    for func in nc.m.functions:
        for block in func.blocks:
            out = []
            for inst in block.instructions:
                si = inst.sync_info
                if si is not None and si.on_wait and len(si.on_wait) > 1:
                    waits = list(si.on_wait)
                    for w in waits[:-1]:
                        n += 1
                        nop = mybir.InstEventSemaphore(
                            name=f"{inst.name}_sw{n}", engine=inst.engine,
                            ins=[], outs=[])
                        nop.sync_info = mybir.SyncInfo(on_wait=[w], on_update=[])
                        out.append(nop)
                    inst.sync_info = mybir.SyncInfo(on_wait=[waits[-1]],
                                                    on_update=list(si.on_update))
                out.append(inst)
            if n:
                block.instructions[:] = out
    return n


def build_nc(split=True):
    nc = bass.Bass("TRN2", target_bir_lowering=False, debug=False)
    pred_d = nc.dram_tensor("predictions", [ROWS, 85], F32, kind="ExternalInput")
    tgt_d = nc.dram_tensor("targets", [NT, 5], F32, kind="ExternalInput")
    out_d = nc.dram_tensor("out", [8, 1], F32, kind="ExternalOutput")

    pred_ap = pred_d.ap()
    n_conf_cols = NCH if CONF_VARIANT == "strided" else 800 // BULK_R

    with tile.TileContext(nc) as tc:
        with tc.tile_pool(name="persist", bufs=1) as pp, \
             tc.tile_pool(name="conf", bufs=NCH) as cp, \
             tc.tile_pool(name="sb", bufs=2) as sb, \
             tc.tile_pool(name="ps", bufs=1, space="PSUM") as ps:

            acc = pp.tile([128, n_conf_cols], F32)

            # constants (route matmul operands through DVE so each matmul
            # needs at most ONE sync wait — the S3_LW slot limit)
            ident_g = pp.tile([128, 128], F32)
            make_identity(nc, ident_g[:])
            ident = pp.tile([128, 128], F32)
            nc.vector.tensor_copy(out=ident[:], in_=ident_g[:])
            ones = pp.tile([128, 1], F32)
            nc.vector.memset(ones[:], 1.0)
            iotac = pp.tile([128, C], I32)
            nc.gpsimd.iota(iotac[:], pattern=[[1, C]], base=0, channel_multiplier=0)
            iotaf = pp.tile([128, C], F32)
            nc.vector.tensor_copy(out=iotaf[:], in_=iotac[:])
            iotap = pp.tile([128, 1], I32)
            nc.gpsimd.iota(iotap[:], pattern=[[1, 1]], base=0, channel_multiplier=1)
            pf128 = pp.tile([128, 1], F32)
            nc.vector.tensor_copy(out=pf128[:], in_=iotap[:])
            iotar = pp.tile([128, 128], I32)
            nc.gpsimd.iota(iotar[:], pattern=[[1, 128]], base=0, channel_multiplier=0)
            iotarf = pp.tile([128, 128], F32)
            nc.vector.tensor_copy(out=iotarf[:], in_=iotar[:])
            tri = pp.tile([128, 128], F32)  # tri[p, f] = 1.0 iff f < p
            nc.vector.tensor_tensor(out=tri[:], in0=pf128[:].to_broadcast([128, 128]),
                                    in1=iotarf[:], op=ALU.is_gt)

            # ---- conf channel: sum ln(1-sigmoid(x)) over all cells
            if CONF_VARIANT == "strided":
                _conf_pass_strided(nc, cp, sb, pred_ap, acc)
            else:
                _conf_pass_bulk(nc, sb, pred_ap, acc)

            # ---- per-target phase: two halves of 100 targets (2 whole batches each)
            P = HALF
            stats_ps = ps.tile([5, 1], F32, space="PSUM")
            for q in range(2):
                tt = sb.tile([P, 5], F32, tag="tt")
                nc.sync.dma_start(out=tt[:], in_=tgt_d.ap()[q * P:(q + 1) * P, :])

                xW = sb.tile([P, 1], F32, tag="xW")
                yH = sb.tile([P, 1], F32, tag="yH")
                nc.vector.tensor_scalar_mul(xW[:], tt[:, 1:2], float(W))
                nc.vector.tensor_scalar_mul(yH[:], tt[:, 2:3], float(H))
                gx = sb.tile([P, 1], F32, tag="gx")
                gy = sb.tile([P, 1], F32, tag="gy")
                _floor(nc, sb, gx[:], xW[:], P)
                _floor(nc, sb, gy[:], yH[:], P)

                # validity
                vf = sb.tile([P, 1], F32, tag="vf")
                tmp = sb.tile([P, 1], F32, tag="tmp")
                nc.vector.tensor_scalar(out=vf[:], in0=gx[:], scalar1=0.0, scalar2=None, op0=ALU.is_ge)
                nc.vector.tensor_scalar(out=tmp[:], in0=gx[:], scalar1=float(W), scalar2=None, op0=ALU.is_lt)
                nc.vector.tensor_tensor(out=vf[:], in0=vf[:], in1=tmp[:], op=ALU.mult)
                nc.vector.tensor_scalar(out=tmp[:], in0=gy[:], scalar1=0.0, scalar2=None, op0=ALU.is_ge)
                nc.vector.tensor_tensor(out=vf[:], in0=vf[:], in1=tmp[:], op=ALU.mult)
                nc.vector.tensor_scalar(out=tmp[:], in0=gy[:], scalar1=float(H), scalar2=None, op0=ALU.is_lt)
                nc.vector.tensor_tensor(out=vf[:], in0=vf[:], in1=tmp[:], op=ALU.mult)

                # cell + per-core row index
                gxi = sb.tile([P, 1], F32, tag="gxi")
                gyi = sb.tile([P, 1], F32, tag="gyi")
                nc.vector.tensor_scalar(out=gxi[:], in0=gx[:], scalar1=0.0, scalar2=float(W - 1),
                                        op0=ALU.max, op1=ALU.min)
                nc.vector.tensor_scalar(out=gyi[:], in0=gy[:], scalar1=0.0, scalar2=float(H - 1),
                                        op0=ALU.max, op1=ALU.min)
                cell = sb.tile([P, 1], F32, tag="cell")
                nc.vector.tensor_scalar_mul(cell[:], gyi[:], float(W))
                nc.vector.tensor_tensor(out=cell[:], in0=cell[:], in1=gxi[:], op=ALU.add)

                rowf = sb.tile([P, 1], F32, tag="rowf")
                # batch offset: (2q + (t>=50)) * HWC
                nc.vector.tensor_scalar(out=rowf[:], in0=pf128[:P, :], scalar1=float(T), scalar2=None,
                                        op0=ALU.is_ge)
                nc.vector.tensor_scalar(out=rowf[:], in0=rowf[:], scalar1=float(HWC),
                                        scalar2=float(2 * q * HWC), op0=ALU.mult, op1=ALU.add)
                nc.vector.tensor_tensor(out=rowf[:], in0=rowf[:], in1=cell[:], op=ALU.add)
                idx = sb.tile([P, 1], I32, tag="idx")
                nc.vector.tensor_copy(out=idx[:], in_=rowf[:])

                # dedup key: valid -> rowf ; invalid -> unique negative
                negk = sb.tile([P, 1], F32, tag="negk")
                nc.vector.tensor_scalar(out=negk[:], in0=pf128[:P, :], scalar1=-1.0,
                                        scalar2=-(1.0 + 100.0 * q), op0=ALU.mult, op1=ALU.add)
                key = sb.tile([P, 1], F32, tag="key")
                nc.vector.tensor_tensor(out=key[:], in0=rowf[:], in1=negk[:], op=ALU.subtract)
                nc.vector.tensor_tensor(out=key[:], in0=key[:], in1=vf[:], op=ALU.mult)
                nc.vector.tensor_tensor(out=key[:], in0=key[:], in1=negk[:], op=ALU.add)

                # gather prediction rows
                rows = sb.tile([P, 85], F32, tag="rows")
                nc.gpsimd.indirect_dma_start(
                    out=rows[:], out_offset=None, in_=pred_ap[:, :],
                    in_offset=IndirectOffsetOnAxis(ap=idx[:, :1], axis=0))

                # sigmoid/ln terms over the whole row
                sg = sb.tile([P, 85], F32, tag="sg")
                nc.scalar.activation(out=sg[:], in_=rows[:], func=AF.Sigmoid)
                lnp = sb.tile([P, 85], F32, tag="lnp")
                nc.scalar.activation(out=lnp[:], in_=sg[:], func=AF.Ln)
                nc.vector.tensor_scalar_max(lnp[:], lnp[:], -100.0)
                om = sb.tile([P, 85], F32, tag="om")
                nc.vector.tensor_scalar(out=om[:], in0=sg[:], scalar1=1.0, scalar2=-1.0,
                                        op0=ALU.subtract, op1=ALU.mult)
                lnn = sb.tile([P, 85], F32, tag="lnn")
                nc.scalar.activation(out=lnn[:], in_=om[:], func=AF.Ln)
                nc.vector.tensor_scalar_max(lnn[:], lnn[:], -100.0)

                # per_cls = -(1/C) * sum_c [ onehot*lnp + (1-onehot)*lnn ]
                oh = sb.tile([P, C], F32, tag="oh")
                nc.vector.tensor_tensor(out=oh[:], in0=iotaf[:P, :],
                                        in1=tt[:, 0:1].to_broadcast([P, C]), op=ALU.is_equal)
                dlt = sb.tile([P, C], F32, tag="dlt")
                nc.vector.tensor_tensor(out=dlt[:], in0=lnp[:, 5:85], in1=lnn[:, 5:85], op=ALU.subtract)
                nc.vector.tensor_tensor(out=dlt[:], in0=dlt[:], in1=oh[:], op=ALU.mult)
                nc.vector.tensor_tensor(out=dlt[:], in0=dlt[:], in1=lnn[:, 5:85], op=ALU.add)
                pcls = sb.tile([P, 1], F32, tag="pcls")
                nc.vector.reduce_sum(out=pcls[:], in_=dlt[:], axis=AX.X)
                nc.vector.tensor_scalar_mul(pcls[:], pcls[:], -1.0 / C)

                # conf correction term: ct = lnn[4] - lnp[4]  ( = term_pos - term_neg )
                ct = sb.tile([P, 1], F32, tag="ct")
                nc.vector.tensor_tensor(out=ct[:], in0=lnn[:, 4:5], in1=lnp[:, 4:5], op=ALU.subtract)

                # per_xy / per_wh
                txy = sb.tile([P, 2], F32, tag="txy")
                nc.vector.tensor_tensor(out=txy[:, 0:1], in0=xW[:], in1=gx[:], op=ALU.subtract)
                nc.vector.tensor_tensor(out=txy[:, 1:2], in0=yH[:], in1=gy[:], op=ALU.subtract)
                dxy = sb.tile([P, 2], F32, tag="dxy")
                nc.vector.tensor_tensor(out=dxy[:], in0=sg[:, 0:2], in1=txy[:], op=ALU.subtract)
                nc.vector.tensor_tensor(out=dxy[:], in0=dxy[:], in1=dxy[:], op=ALU.mult)
                pxy = sb.tile([P, 1], F32, tag="pxy")
                nc.vector.reduce_sum(out=pxy[:], in_=dxy[:], axis=AX.X)
                nc.vector.tensor_scalar_mul(pxy[:], pxy[:], 0.5)

                pwh_t = sb.tile([P, 2], F32, tag="pwh")
                nc.scalar.activation(out=pwh_t[:], in_=rows[:, 2:4], func=AF.Exp)
                twh = sb.tile([P, 2], F32, tag="twh")
                nc.vector.tensor_scalar_mul(twh[:, 0:1], tt[:, 3:4], float(W))
                nc.vector.tensor_scalar_mul(twh[:, 1:2], tt[:, 4:5], float(H))
                dwh = sb.tile([P, 2], F32, tag="dwh")
                nc.vector.tensor_tensor(out=dwh[:], in0=pwh_t[:], in1=twh[:], op=ALU.subtract)
                nc.vector.tensor_tensor(out=dwh[:], in0=dwh[:], in1=dwh[:], op=ALU.mult)
                pwh = sb.tile([P, 1], F32, tag="pwh1")
                nc.vector.reduce_sum(out=pwh[:], in_=dwh[:], axis=AX.X)
                nc.vector.tensor_scalar_mul(pwh[:], pwh[:], 0.5)

                # dedup: first-occurrence weight w
                keyT_ps = ps.tile([P, P], F32, space="PSUM", tag="keyT_ps")
                nc.tensor.transpose(out=keyT_ps[:], in_=key[:].to_broadcast([P, P]),
                                    identity=ident[:P, :P])
                keyT = sb.tile([P, P], F32, tag="keyT")
                nc.vector.tensor_copy(out=keyT[:], in_=keyT_ps[:])
                eq = sb.tile([P, P], F32, tag="eq")
                nc.vector.tensor_tensor(out=eq[:], in0=key[:].to_broadcast([P, P]),
                                        in1=keyT[:], op=ALU.is_equal)
                nc.vector.tensor_tensor(out=eq[:], in0=eq[:], in1=tri[:P, :P], op=ALU.mult)
                dup = sb.tile([P, 1], F32, tag="dup")
                nc.vector.reduce_max(out=dup[:], in_=eq[:], axis=AX.X)
                wfo = sb.tile([P, 1], F32, tag="wfo")
                nc.vector.tensor_scalar(out=wfo[:], in0=dup[:], scalar1=-1.0, scalar2=1.0,
                                        op0=ALU.mult, op1=ALU.add)
                nc.vector.tensor_tensor(out=wfo[:], in0=wfo[:], in1=vf[:], op=ALU.mult)

                # stats columns: vf*pxy, vf*pwh, vf*pcls, vf, w*ct
                stats = sb.tile([P, 5], F32, tag="stats")
                nc.vector.tensor_tensor(out=stats[:, 0:1], in0=pxy[:], in1=vf[:], op=ALU.mult)
                nc.vector.tensor_tensor(out=stats[:, 1:2], in0=pwh[:], in1=vf[:], op=ALU.mult)
                nc.vector.tensor_tensor(out=stats[:, 2:3], in0=pcls[:], in1=vf[:], op=ALU.mult)
                nc.vector.tensor_copy(out=stats[:, 3:4], in_=vf[:])
                nc.vector.tensor_tensor(out=stats[:, 4:5], in0=ct[:], in1=wfo[:], op=ALU.mult)

                nc.tensor.matmul(out=stats_ps[:], lhsT=stats[:], rhs=ones[:P, :],
                                 start=(q == 0), stop=(q == 1))

            # ---- final reductions
            racc = pp.tile([128, 1], F32)
            nc.vector.reduce_sum(out=racc[:], in_=acc[:], axis=AX.X)
            conf_ps = ps.tile([1, 1], F32, space="PSUM")
            nc.tensor.matmul(out=conf_ps[:], lhsT=ones[:], rhs=racc[:], start=True, stop=True)

            so = pp.tile([5, 1], F32)
            nc.vector.tensor_copy(out=so[:], in_=stats_ps[:])
            co = pp.tile([1, 1], F32)
            nc.vector.tensor_copy(out=co[:], in_=conf_ps[:])
            nc.gpsimd.dma_start(out=out_d.ap()[0:5, :], in_=so[:])
            nc.gpsimd.dma_start(out=out_d.ap()[5:6, :], in_=co[:])
    if split:
        _split_multi_waits(nc)
    return nc


_NC_CACHE = None


def _get_nc():
    global _NC_CACHE
    if _NC_CACHE is None:
        _NC_CACHE = build_nc()
    return _NC_CACHE


def make_in_maps(predictions, targets):
    preds = np.ascontiguousarray(np.asarray(predictions, dtype=np.float32)).reshape(NCORES, ROWS, 85)
    tgts = np.ascontiguousarray(np.asarray(targets, dtype=np.float32)).reshape(NCORES, NT, 5)
    return [{"predictions": preds[c], "targets": tgts[c]} for c in range(NCORES)]


def combine_partials(parts):
    """parts: list of 8 arrays [8,1] -> (total, loss_xy, loss_wh, loss_conf, loss_cls)"""
    s = np.sum([p.reshape(-1) for p in parts], axis=0, dtype=np.float64)
    xy, wh, cls_, nt, corr, lnsum = [np.float32(v) for v in s[:6]]
    denom = np.float32(max(float(nt), 1.0))
    loss_xy = np.float32(xy / denom)
    loss_wh = np.float32(wh / denom)
    loss_cls = np.float32(cls_ / denom)
    loss_conf = np.float32((-lnsum + corr) / np.float32(B * HWC))
    total = np.float32(5.0 * loss_xy + 5.0 * loss_wh + loss_conf + loss_cls)
    return total, loss_xy, loss_wh, loss_conf, loss_cls


def kernel(predictions, targets, H=None, W=None):
    from concourse.bass_utils import run_bass_kernel_spmd

    nc = _get_nc()
    in_maps = make_in_maps(predictions, targets)
    res = run_bass_kernel_spmd(nc, in_maps, core_ids=list(range(NCORES)))
    parts = [res.results[c]["out"] for c in range(NCORES)]
    return combine_partials(parts)



# revision 5
# speedup vs baseline: 1.2320x; 1.1840x over previous
"""Trainium2 Bass kernel for nn_MinimalLoss (YOLO-style detection loss).

Strategy (data-parallel over 8 NeuronCores, 4 batches each):
  * conf channel: 102400 4-byte strided gathers/core. This is a hard
    ~62us floor (16 SDMA engines x ~9.6ns/descriptor). The Sync engine
    is dedicated to issuing these 8 chunk DMAs back-to-back, starting
    as the very first kernel instruction.
  * everything else (per-target gather + losses, dedup, constants) is
    issued on gpsimd/DVE/ACT/PE and hides completely under the conf DMA.
  * all activation math uses only Exp/Ln (one ACT table, zero 1.28us
    table swaps) via softplus identities:
       ln sigma(x)      = -ln(1+e^-x)
       ln(1-sigma(x))   = -x - ln(1+e^-x)
       sigma(x)         = exp(-ln(1+e^-x))
       lnn - lnp        = -x      (conf correction term is just -logit)
       sum_cells ln(1-sigma) = -sum softplus = -sum ln(1+e^x)
  * final reduction on host: per-core outputs are acc [128, NCH]
    (per-partition softplus partial sums) and stats [100, 10]
    (per-target columns for both halves).
"""
import numpy as np

import concourse.bass as bass
import concourse.mybir as mybir
import concourse.tile as tile
from concourse.bass import IndirectOffsetOnAxis
from concourse.masks import make_identity

F32 = mybir.dt.float32
I32 = mybir.dt.int32
AF = mybir.ActivationFunctionType
ALU = mybir.AluOpType
AX = mybir.AxisListType

B, HWC, C, T = 32, 25600, 80, 50          # full problem
H = W = 160
NCORES = 8
BL = B // NCORES                          # 4 batches per core
ROWS = BL * HWC                           # 102400 prediction rows per core
NT = BL * T                               # 200 targets per core
HALF = NT // 2                            # 100 targets per half (2 batches)
MAGIC = float(np.float32(2 ** 23))
NCH = 8                                   # conf chunks
CW = 800 // NCH                           # conf cols per chunk


def _floor(nc, sb, dst, src, n):
    """dst = floor(src) for 0 <= src < 2^22, exact (round-to-nearest fixup)."""
    r = sb.tile([n, 1], F32, tag="fl_r")
    adj = sb.tile([n, 1], F32, tag="fl_a")
    nc.vector.tensor_scalar_add(r[:], src, MAGIC)
    nc.vector.tensor_scalar_add(r[:], r[:], -MAGIC)
    nc.vector.tensor_tensor(out=adj[:], in0=r[:], in1=src, op=ALU.is_gt)
    nc.vector.tensor_tensor(out=dst, in0=r[:], in1=adj[:], op=ALU.subtract)


def _split_multi_waits(nc):
    """Walrus codegen accepts at most ONE sync wait per instruction; hoist
    extras onto standalone EventSemaphore (wait) ops on the same engine."""
    n = 0
    for func in nc.m.functions:
        for block in func.blocks:
            out = []
            for inst in block.instructions:
                si = inst.sync_info
                if si is not None and si.on_wait and len(si.on_wait) > 1:
                    waits = list(si.on_wait)
                    for w in waits[:-1]:
                        n += 1
                        nop = mybir.InstEventSemaphore(
                            name=f"{inst.name}_sw{n}", engine=inst.engine,
                            ins=[], outs=[])
                        nop.sync_info = mybir.SyncInfo(on_wait=[w], on_update=[])
                        out.append(nop)
                    inst.sync_info = mybir.SyncInfo(on_wait=[waits[-1]],
                                                    on_update=list(si.on_update))
                out.append(inst)
            if n:
                block.instructions[:] = out
    return n


def build_nc(split=True):
    nc = bass.Bass("TRN2", target_bir_lowering=False, debug=False)
    pred_d = nc.dram_tensor("predictions", [ROWS, 85], F32, kind="ExternalInput")
    tgt_d = nc.dram_tensor("targets", [NT, 5], F32, kind="ExternalInput")
    stats_d = nc.dram_tensor("stats", [HALF, 10], F32, kind="ExternalOutput")
    acc_d = nc.dram_tensor("acc", [128, NCH], F32, kind="ExternalOutput")

    pred_ap = pred_d.ap()

    with tile.TileContext(nc) as tc:
        with tc.tile_pool(name="persist", bufs=1) as pp, \
             tc.tile_pool(name="conf", bufs=NCH) as cp, \
             tc.tile_pool(name="sb", bufs=2) as sb, \
             tc.tile_pool(name="ps", bufs=1, space="PSUM") as ps:

            # ---- conf channel DMAs: the critical path. Sync engine does
            # nothing else; issues stream back-to-back from t~=0.
            conf = pred_ap[:, 4:5].rearrange("(p j) o -> p (j o)", p=128)  # [128, 800]
            conf_tl = []
            for k in range(NCH):
                tl = cp.tile([128, CW], F32, tag="conf_in")
                nc.sync.dma_start(out=tl[:], in_=conf[:, k * CW:(k + 1) * CW])
                conf_tl.append(tl)

            accT = pp.tile([128, NCH], F32)

            # ---- targets load early (gpsimd queue, independent of conf)
            # [100, 10]: half q in cols 5q..5q+4
            tt2 = pp.tile([HALF, 10], F32)
            nc.gpsimd.dma_start(out=tt2[:, 0:5], in_=tgt_d.ap()[0:HALF, :])
            nc.gpsimd.dma_start(out=tt2[:, 5:10], in_=tgt_d.ap()[HALF:NT, :])

            # ---- constants (gpsimd iota + DVE copies; matmul operands routed
            # through DVE so each PE op needs at most ONE sync wait)
            ident_g = pp.tile([128, 128], F32)
            make_identity(nc, ident_g[:])
            ident = pp.tile([128, 128], F32)
            nc.vector.tensor_copy(out=ident[:], in_=ident_g[:])
            iotac = pp.tile([128, C], I32)
            nc.gpsimd.iota(iotac[:], pattern=[[1, C]], base=0, channel_multiplier=0)
            iotaf = pp.tile([128, C], F32)
            nc.vector.tensor_copy(out=iotaf[:], in_=iotac[:])
            iotap = pp.tile([128, 1], I32)
            nc.gpsimd.iota(iotap[:], pattern=[[1, 1]], base=0, channel_multiplier=1)
            pf128 = pp.tile([128, 1], F32)
            nc.vector.tensor_copy(out=pf128[:], in_=iotap[:])
            iotar = pp.tile([128, 128], I32)
            nc.gpsimd.iota(iotar[:], pattern=[[1, 128]], base=0, channel_multiplier=0)
            iotarf = pp.tile([128, 128], F32)
            nc.vector.tensor_copy(out=iotarf[:], in_=iotar[:])
            tri = pp.tile([128, 128], F32)  # tri[p, f] = 1.0 iff f < p
            nc.vector.tensor_tensor(out=tri[:], in0=pf128[:].to_broadcast([128, 128]),
                                    in1=iotarf[:], op=ALU.is_gt)

            stats2 = pp.tile([HALF, 10], F32)

            # ---- per-target phase: two halves of 100 targets (2 whole batches
            # each), entirely hidden under the conf DMA stream.
            P = HALF
            for q in range(2):
                o = 5 * q
                xW = sb.tile([P, 1], F32, tag="xW")
                yH = sb.tile([P, 1], F32, tag="yH")
                nc.vector.tensor_scalar_mul(xW[:], tt2[:, o + 1:o + 2], float(W))
                nc.vector.tensor_scalar_mul(yH[:], tt2[:, o + 2:o + 3], float(H))
                gx = sb.tile([P, 1], F32, tag="gx")
                gy = sb.tile([P, 1], F32, tag="gy")
                _floor(nc, sb, gx[:], xW[:], P)
                _floor(nc, sb, gy[:], yH[:], P)

                # validity
                vf = sb.tile([P, 1], F32, tag="vf")
                tmp = sb.tile([P, 1], F32, tag="tmp")
                nc.vector.tensor_scalar(out=vf[:], in0=gx[:], scalar1=0.0, scalar2=None, op0=ALU.is_ge)
                nc.vector.tensor_scalar(out=tmp[:], in0=gx[:], scalar1=float(W), scalar2=None, op0=ALU.is_lt)
                nc.vector.tensor_tensor(out=vf[:], in0=vf[:], in1=tmp[:], op=ALU.mult)
                nc.vector.tensor_scalar(out=tmp[:], in0=gy[:], scalar1=0.0, scalar2=None, op0=ALU.is_ge)
                nc.vector.tensor_tensor(out=vf[:], in0=vf[:], in1=tmp[:], op=ALU.mult)
                nc.vector.tensor_scalar(out=tmp[:], in0=gy[:], scalar1=float(H), scalar2=None, op0=ALU.is_lt)
                nc.vector.tensor_tensor(out=vf[:], in0=vf[:], in1=tmp[:], op=ALU.mult)

                # cell + per-core row index
                gxi = sb.tile([P, 1], F32, tag="gxi")
                gyi = sb.tile([P, 1], F32, tag="gyi")
                nc.vector.tensor_scalar(out=gxi[:], in0=gx[:], scalar1=0.0, scalar2=float(W - 1),
                                        op0=ALU.max, op1=ALU.min)
                nc.vector.tensor_scalar(out=gyi[:], in0=gy[:], scalar1=0.0, scalar2=float(H - 1),
                                        op0=ALU.max, op1=ALU.min)
                cell = sb.tile([P, 1], F32, tag="cell")
                nc.vector.tensor_scalar_mul(cell[:], gyi[:], float(W))
                nc.vector.tensor_tensor(out=cell[:], in0=cell[:], in1=gxi[:], op=ALU.add)

                rowf = sb.tile([P, 1], F32, tag="rowf")
                # batch offset: (2q + (t>=50)) * HWC
                nc.vector.tensor_scalar(out=rowf[:], in0=pf128[:P, :], scalar1=float(T), scalar2=None,
                                        op0=ALU.is_ge)
                nc.vector.tensor_scalar(out=rowf[:], in0=rowf[:], scalar1=float(HWC),
                                        scalar2=float(2 * q * HWC), op0=ALU.mult, op1=ALU.add)
                nc.vector.tensor_tensor(out=rowf[:], in0=rowf[:], in1=cell[:], op=ALU.add)
                idx = sb.tile([P, 1], I32, tag="idx")
                nc.vector.tensor_copy(out=idx[:], in_=rowf[:])

                # dedup key: valid -> rowf ; invalid -> unique negative
                negk = sb.tile([P, 1], F32, tag="negk")
                nc.vector.tensor_scalar(out=negk[:], in0=pf128[:P, :], scalar1=-1.0,
                                        scalar2=-(1.0 + 100.0 * q), op0=ALU.mult, op1=ALU.add)
                key = sb.tile([P, 1], F32, tag="key")
                nc.vector.tensor_tensor(out=key[:], in0=rowf[:], in1=negk[:], op=ALU.subtract)
                nc.vector.tensor_tensor(out=key[:], in0=key[:], in1=vf[:], op=ALU.mult)
                nc.vector.tensor_tensor(out=key[:], in0=key[:], in1=negk[:], op=ALU.add)

                # gather prediction rows
                rows = sb.tile([P, 85], F32, tag="rows")
                nc.gpsimd.indirect_dma_start(
                    out=rows[:], out_offset=None, in_=pred_ap[:, :],
                    in_offset=IndirectOffsetOnAxis(ap=idx[:, :1], axis=0))

                # cls: bce_sum = sum_c(x_c + s_c) - x_{c*},  s_c = ln(1+e^-x_c)
                ecls = sb.tile([P, C], F32, tag="ecls")
                nc.scalar.activation(out=ecls[:], in_=rows[:, 5:85], func=AF.Exp, scale=-1.0)
                nc.vector.tensor_scalar_add(ecls[:], ecls[:], 1.0)
                scls = sb.tile([P, C], F32, tag="scls")
                ssum = sb.tile([P, 1], F32, tag="ssum")
                nc.scalar.activation(out=scls[:], in_=ecls[:], func=AF.Ln, accum_out=ssum[:])
                xsum = sb.tile([P, 1], F32, tag="xsum")
                nc.vector.reduce_sum(out=xsum[:], in_=rows[:, 5:85], axis=AX.X)
                oh = sb.tile([P, C], F32, tag="oh")
                nc.vector.tensor_tensor(out=oh[:], in0=iotaf[:P, :],
                                        in1=tt2[:, o:o + 1].to_broadcast([P, C]), op=ALU.is_equal)
                ohx = sb.tile([P, C], F32, tag="ohx")
                nc.vector.tensor_tensor(out=ohx[:], in0=oh[:], in1=rows[:, 5:85], op=ALU.mult)
                xstar = sb.tile([P, 1], F32, tag="xstar")
                nc.vector.reduce_sum(out=xstar[:], in_=ohx[:], axis=AX.X)
                pcls = sb.tile([P, 1], F32, tag="pcls")
                nc.vector.tensor_tensor(out=pcls[:], in0=ssum[:], in1=xsum[:], op=ALU.add)
                nc.vector.tensor_tensor(out=pcls[:], in0=pcls[:], in1=xstar[:], op=ALU.subtract)
                nc.vector.tensor_scalar_mul(pcls[:], pcls[:], 1.0 / C)

                # xy: sigma(x) = exp(-ln(1+e^-x))
                exy = sb.tile([P, 2], F32, tag="exy")
                nc.scalar.activation(out=exy[:], in_=rows[:, 0:2], func=AF.Exp, scale=-1.0)
                nc.vector.tensor_scalar_add(exy[:], exy[:], 1.0)
                sxy = sb.tile([P, 2], F32, tag="sxy")
                nc.scalar.activation(out=sxy[:], in_=exy[:], func=AF.Ln)
                sigxy = sb.tile([P, 2], F32, tag="sigxy")
                nc.scalar.activation(out=sigxy[:], in_=sxy[:], func=AF.Exp, scale=-1.0)
                txy = sb.tile([P, 2], F32, tag="txy")
                nc.vector.tensor_tensor(out=txy[:, 0:1], in0=xW[:], in1=gx[:], op=ALU.subtract)
                nc.vector.tensor_tensor(out=txy[:, 1:2], in0=yH[:], in1=gy[:], op=ALU.subtract)
                dxy = sb.tile([P, 2], F32, tag="dxy")
                nc.vector.tensor_tensor(out=dxy[:], in0=sigxy[:], in1=txy[:], op=ALU.subtract)
                nc.vector.tensor_tensor(out=dxy[:], in0=dxy[:], in1=dxy[:], op=ALU.mult)
                pxy = sb.tile([P, 1], F32, tag="pxy")
                nc.vector.reduce_sum(out=pxy[:], in_=dxy[:], axis=AX.X)
                nc.vector.tensor_scalar_mul(pxy[:], pxy[:], 0.5)

                # wh
                pwh_t = sb.tile([P, 2], F32, tag="pwh")
                nc.scalar.activation(out=pwh_t[:], in_=rows[:, 2:4], func=AF.Exp)
                twh = sb.tile([P, 2], F32, tag="twh")
                nc.vector.tensor_scalar_mul(twh[:, 0:1], tt2[:, o + 3:o + 4], float(W))
                nc.vector.tensor_scalar_mul(twh[:, 1:2], tt2[:, o + 4:o + 5], float(H))
                dwh = sb.tile([P, 2], F32, tag="dwh")
                nc.vector.tensor_tensor(out=dwh[:], in0=pwh_t[:], in1=twh[:], op=ALU.subtract)
                nc.vector.tensor_tensor(out=dwh[:], in0=dwh[:], in1=dwh[:], op=ALU.mult)
                pwh = sb.tile([P, 1], F32, tag="pwh1")
                nc.vector.reduce_sum(out=pwh[:], in_=dwh[:], axis=AX.X)
                nc.vector.tensor_scalar_mul(pwh[:], pwh[:], 0.5)

                # dedup: first-occurrence weight w (for obj_mask scatter-max)
                keyT_ps = ps.tile([P, P], F32, space="PSUM", tag="keyT_ps")
                nc.tensor.transpose(out=keyT_ps[:], in_=key[:].to_broadcast([P, P]),
                                    identity=ident[:P, :P])
                keyT = sb.tile([P, P], F32, tag="keyT")
                nc.vector.tensor_copy(out=keyT[:], in_=keyT_ps[:])
                eq = sb.tile([P, P], F32, tag="eq")
                nc.vector.tensor_tensor(out=eq[:], in0=key[:].to_broadcast([P, P]),
                                        in1=keyT[:], op=ALU.is_equal)
                nc.vector.tensor_tensor(out=eq[:], in0=eq[:], in1=tri[:P, :P], op=ALU.mult)
                dup = sb.tile([P, 1], F32, tag="dup")
                nc.vector.reduce_max(out=dup[:], in_=eq[:], axis=AX.X)
                wfo = sb.tile([P, 1], F32, tag="wfo")
                nc.vector.tensor_scalar(out=wfo[:], in0=dup[:], scalar1=-1.0, scalar2=1.0,
                                        op0=ALU.mult, op1=ALU.add)
                nc.vector.tensor_tensor(out=wfo[:], in0=wfo[:], in1=vf[:], op=ALU.mult)

                # stats columns: vf*pxy, vf*pwh, vf*pcls, vf, wfo*x4 (host negates)
                nc.vector.tensor_tensor(out=stats2[:, o + 0:o + 1], in0=pxy[:], in1=vf[:], op=ALU.mult)
                nc.vector.tensor_tensor(out=stats2[:, o + 1:o + 2], in0=pwh[:], in1=vf[:], op=ALU.mult)
                nc.vector.tensor_tensor(out=stats2[:, o + 2:o + 3], in0=pcls[:], in1=vf[:], op=ALU.mult)
                nc.vector.tensor_copy(out=stats2[:, o + 3:o + 4], in_=vf[:])
                nc.vector.tensor_tensor(out=stats2[:, o + 4:o + 5], in0=rows[:, 4:5], in1=wfo[:], op=ALU.mult)

            # ---- conf compute: softplus(x) = ln(1+e^x) summed per partition
            for k in range(NCH):
                ex = cp.tile([128, CW], F32, tag="conf_ex")
                nc.scalar.activation(out=ex[:], in_=conf_tl[k][:], func=AF.Exp)
                nc.vector.tensor_scalar_add(ex[:], ex[:], 1.0)
                lnt = cp.tile([128, CW], F32, tag="conf_ln")
                nc.scalar.activation(out=lnt[:], in_=ex[:], func=AF.Ln,
                                     accum_out=accT[:, k:k + 1])

            # ---- outputs (sync stream positions 9 & 10, after conf issues)
            nc.sync.dma_start(out=stats_d.ap()[:, :], in_=stats2[:])
            nc.sync.dma_start(out=acc_d.ap()[:, :], in_=accT[:])
    if split:
        _split_multi_waits(nc)
    return nc


_NC_CACHE = None


def _get_nc():
    global _NC_CACHE
    if _NC_CACHE is None:
        _NC_CACHE = build_nc()
    return _NC_CACHE


def make_in_maps(predictions, targets):
    preds = np.ascontiguousarray(np.asarray(predictions, dtype=np.float32)).reshape(NCORES, ROWS, 85)
    tgts = np.ascontiguousarray(np.asarray(targets, dtype=np.float32)).reshape(NCORES, NT, 5)
    return [{"predictions": preds[c], "targets": tgts[c]} for c in range(NCORES)]


def combine_partials(results):
    """results: list of 8 dicts with 'stats' [100,10] and 'acc' [128,NCH]
    -> (total, loss_xy, loss_wh, loss_conf, loss_cls)"""
    st = np.sum([np.asarray(r["stats"], dtype=np.float64) for r in results], axis=(0, 1))
    sp_total = float(np.sum([np.asarray(r["acc"], dtype=np.float64) for r in results]))
    xy = st[0] + st[5]
    wh = st[1] + st[6]
    cls_ = st[2] + st[7]
    nt = st[3] + st[8]
    x4 = st[4] + st[9]
    denom = np.float32(max(float(nt), 1.0))
    loss_xy = np.float32(np.float32(xy) / denom)
    loss_wh = np.float32(np.float32(wh) / denom)
    loss_cls = np.float32(np.float32(cls_) / denom)
    loss_conf = np.float32((np.float32(sp_total) - np.float32(x4)) / np.float32(B * HWC))
    total = np.float32(5.0 * loss_xy + 5.0 * loss_wh + loss_conf + loss_cls)
    return total, loss_xy, loss_wh, loss_conf, loss_cls


def kernel(predictions, targets, H=None, W=None):
    from concourse.bass_utils import run_bass_kernel_spmd

    nc = _get_nc()
    in_maps = make_in_maps(predictions, targets)
    res = run_bass_kernel_spmd(nc, in_maps, core_ids=list(range(NCORES)))
    return combine_partials([res.results[c] for c in range(NCORES)])


# revision 7
# speedup vs baseline: 1.3059x; 1.0600x over previous
"""Trainium2 Bass kernel for nn_MinimalLoss (YOLO-style detection loss).

Strategy (data-parallel over 8 NeuronCores, 4 batches each):
  * conf channel: 102400 4-byte strided gathers/core. This is a hard
    ~62us floor (16 SDMA engines x ~9.6ns/descriptor). The Sync engine
    is dedicated to issuing these 8 chunk DMAs back-to-back, starting
    as the very first kernel instruction.
  * everything else (per-target gather + losses, dedup, constants) is
    issued on gpsimd/DVE/ACT/PE and hides completely under the conf DMA.
  * all activation math uses only Exp/Ln (one ACT table, zero 1.28us
    table swaps) via softplus identities:
       ln sigma(x)      = -ln(1+e^-x)
       ln(1-sigma(x))   = -x - ln(1+e^-x)
       sigma(x)         = exp(-ln(1+e^-x))
       lnn - lnp        = -x      (conf correction term is just -logit)
       sum_cells ln(1-sigma) = -sum softplus = -sum ln(1+e^x)
  * final reduction on host: per-core outputs are acc [128, NCH]
    (per-partition softplus partial sums) and stats [100, 10]
    (per-target columns for both halves).
"""
import numpy as np

import concourse.bass as bass
import concourse.mybir as mybir
import concourse.tile as tile
from concourse.bass import IndirectOffsetOnAxis
from concourse.masks import make_identity

F32 = mybir.dt.float32
I32 = mybir.dt.int32
AF = mybir.ActivationFunctionType
ALU = mybir.AluOpType
AX = mybir.AxisListType

B, HWC, C, T = 32, 25600, 80, 50          # full problem
H = W = 160
NCORES = 8
BL = B // NCORES                          # 4 batches per core
ROWS = BL * HWC                           # 102400 prediction rows per core
NT = BL * T                               # 200 targets per core
HALF = NT // 2                            # 100 targets per half (2 batches)
MAGIC = float(np.float32(2 ** 23))
NCH = 8                                   # conf chunks
CW = 800 // NCH                           # conf cols per chunk


def _floor(nc, sb, dst, src, n):
    """dst = floor(src) for 0 <= src < 2^22, exact (round-to-nearest fixup)."""
    r = sb.tile([n, 1], F32, tag="fl_r")
    adj = sb.tile([n, 1], F32, tag="fl_a")
    nc.vector.tensor_scalar_add(r[:], src, MAGIC)
    nc.vector.tensor_scalar_add(r[:], r[:], -MAGIC)
    nc.vector.tensor_tensor(out=adj[:], in0=r[:], in1=src, op=ALU.is_gt)
    nc.vector.tensor_tensor(out=dst, in0=r[:], in1=adj[:], op=ALU.subtract)


def _split_multi_waits(nc):
    """Walrus codegen accepts at most ONE sync wait per instruction; hoist
    extras onto standalone EventSemaphore (wait) ops on the same engine."""
    n = 0
    for func in nc.m.functions:
        for block in func.blocks:
            out = []
            for inst in block.instructions:
                si = inst.sync_info
                if si is not None and si.on_wait and len(si.on_wait) > 1:
                    waits = list(si.on_wait)
                    for w in waits[:-1]:
                        n += 1
                        nop = mybir.InstEventSemaphore(
                            name=f"{inst.name}_sw{n}", engine=inst.engine,
                            ins=[], outs=[])
                        nop.sync_info = mybir.SyncInfo(on_wait=[w], on_update=[])
                        out.append(nop)
                    inst.sync_info = mybir.SyncInfo(on_wait=[waits[-1]],
                                                    on_update=list(si.on_update))
                out.append(inst)
            if n:
                block.instructions[:] = out
    return n


def build_nc(split=True):
    nc = bass.Bass("TRN2", target_bir_lowering=False, debug=False)
    pred_d = nc.dram_tensor("predictions", [ROWS, 85], F32, kind="ExternalInput")
    tgt_d = nc.dram_tensor("targets", [NT, 5], F32, kind="ExternalInput")
    stats_d = nc.dram_tensor("stats", [HALF, 10], F32, kind="ExternalOutput")
    acc_d = nc.dram_tensor("acc", [128, NCH], F32, kind="ExternalOutput")

    pred_ap = pred_d.ap()

    with tile.TileContext(nc) as tc:
        with tc.tile_pool(name="persist", bufs=1) as pp, \
             tc.tile_pool(name="conf", bufs=NCH) as cp, \
             tc.tile_pool(name="sb", bufs=2) as sb, \
             tc.tile_pool(name="ps", bufs=1, space="PSUM") as ps:

            # ---- conf channel DMAs: the critical path. Sync engine does
            # nothing else; issues stream back-to-back from t~=0.
            conf = pred_ap[:, 4:5].rearrange("(p j) o -> p (j o)", p=128)  # [128, 800]
            conf_tl = []
            for k in range(NCH):
                tl = cp.tile([128, CW], F32, tag="conf_in")
                nc.sync.dma_start(out=tl[:], in_=conf[:, k * CW:(k + 1) * CW])
                conf_tl.append(tl)

            accT = pp.tile([128, NCH], F32)

            # ---- targets load early (gpsimd queue, independent of conf)
            # [100, 10]: half q in cols 5q..5q+4
            tt2 = pp.tile([HALF, 10], F32)
            nc.gpsimd.dma_start(out=tt2[:, 0:5], in_=tgt_d.ap()[0:HALF, :])
            nc.gpsimd.dma_start(out=tt2[:, 5:10], in_=tgt_d.ap()[HALF:NT, :])

            # ---- constants (gpsimd iota + DVE copies; matmul operands routed
            # through DVE so each PE op needs at most ONE sync wait)
            ident_g = pp.tile([128, 128], F32)
            make_identity(nc, ident_g[:])
            ident = pp.tile([128, 128], F32)
            nc.vector.tensor_copy(out=ident[:], in_=ident_g[:])
            iotac = pp.tile([128, C], I32)
            nc.gpsimd.iota(iotac[:], pattern=[[1, C]], base=0, channel_multiplier=0)
            iotaf = pp.tile([128, C], F32)
            nc.vector.tensor_copy(out=iotaf[:], in_=iotac[:])
            iotap = pp.tile([128, 1], I32)
            nc.gpsimd.iota(iotap[:], pattern=[[1, 1]], base=0, channel_multiplier=1)
            pf128 = pp.tile([128, 1], F32)
            nc.vector.tensor_copy(out=pf128[:], in_=iotap[:])
            iotar = pp.tile([128, 128], I32)
            nc.gpsimd.iota(iotar[:], pattern=[[1, 128]], base=0, channel_multiplier=0)
            iotarf = pp.tile([128, 128], F32)
            nc.vector.tensor_copy(out=iotarf[:], in_=iotar[:])
            tri = pp.tile([128, 128], F32)  # tri[p, f] = 1.0 iff f < p
            nc.vector.tensor_tensor(out=tri[:], in0=pf128[:].to_broadcast([128, 128]),
                                    in1=iotarf[:], op=ALU.is_gt)

            stats2 = pp.tile([HALF, 10], F32)

            # ---- per-target phase: two halves of 100 targets (2 whole batches
            # each), entirely hidden under the conf DMA stream.
            P = HALF
            for q in range(2):
                o = 5 * q
                xW = sb.tile([P, 1], F32, tag="xW")
                yH = sb.tile([P, 1], F32, tag="yH")
                nc.vector.tensor_scalar_mul(xW[:], tt2[:, o + 1:o + 2], float(W))
                nc.vector.tensor_scalar_mul(yH[:], tt2[:, o + 2:o + 3], float(H))
                gx = sb.tile([P, 1], F32, tag="gx")
                gy = sb.tile([P, 1], F32, tag="gy")
                _floor(nc, sb, gx[:], xW[:], P)
                _floor(nc, sb, gy[:], yH[:], P)

                # validity
                vf = sb.tile([P, 1], F32, tag="vf")
                tmp = sb.tile([P, 1], F32, tag="tmp")
                nc.vector.tensor_scalar(out=vf[:], in0=gx[:], scalar1=0.0, scalar2=None, op0=ALU.is_ge)
                nc.vector.tensor_scalar(out=tmp[:], in0=gx[:], scalar1=float(W), scalar2=None, op0=ALU.is_lt)
                nc.vector.tensor_tensor(out=vf[:], in0=vf[:], in1=tmp[:], op=ALU.mult)
                nc.vector.tensor_scalar(out=tmp[:], in0=gy[:], scalar1=0.0, scalar2=None, op0=ALU.is_ge)
                nc.vector.tensor_tensor(out=vf[:], in0=vf[:], in1=tmp[:], op=ALU.mult)
                nc.vector.tensor_scalar(out=tmp[:], in0=gy[:], scalar1=float(H), scalar2=None, op0=ALU.is_lt)
                nc.vector.tensor_tensor(out=vf[:], in0=vf[:], in1=tmp[:], op=ALU.mult)

                # cell + per-core row index
                gxi = sb.tile([P, 1], F32, tag="gxi")
                gyi = sb.tile([P, 1], F32, tag="gyi")
                nc.vector.tensor_scalar(out=gxi[:], in0=gx[:], scalar1=0.0, scalar2=float(W - 1),
                                        op0=ALU.max, op1=ALU.min)
                nc.vector.tensor_scalar(out=gyi[:], in0=gy[:], scalar1=0.0, scalar2=float(H - 1),
                                        op0=ALU.max, op1=ALU.min)
                cell = sb.tile([P, 1], F32, tag="cell")
                nc.vector.tensor_scalar_mul(cell[:], gyi[:], float(W))
                nc.vector.tensor_tensor(out=cell[:], in0=cell[:], in1=gxi[:], op=ALU.add)

                rowf = sb.tile([P, 1], F32, tag="rowf")
                # batch offset: (2q + (t>=50)) * HWC
                nc.vector.tensor_scalar(out=rowf[:], in0=pf128[:P, :], scalar1=float(T), scalar2=None,
                                        op0=ALU.is_ge)
                nc.vector.tensor_scalar(out=rowf[:], in0=rowf[:], scalar1=float(HWC),
                                        scalar2=float(2 * q * HWC), op0=ALU.mult, op1=ALU.add)
                nc.vector.tensor_tensor(out=rowf[:], in0=rowf[:], in1=cell[:], op=ALU.add)
                idx = sb.tile([P, 1], I32, tag="idx")
                nc.vector.tensor_copy(out=idx[:], in_=rowf[:])

                # dedup key: valid -> rowf ; invalid -> unique negative
                negk = sb.tile([P, 1], F32, tag="negk")
                nc.vector.tensor_scalar(out=negk[:], in0=pf128[:P, :], scalar1=-1.0,
                                        scalar2=-(1.0 + 100.0 * q), op0=ALU.mult, op1=ALU.add)
                key = sb.tile([P, 1], F32, tag="key")
                nc.vector.tensor_tensor(out=key[:], in0=rowf[:], in1=negk[:], op=ALU.subtract)
                nc.vector.tensor_tensor(out=key[:], in0=key[:], in1=vf[:], op=ALU.mult)
                nc.vector.tensor_tensor(out=key[:], in0=key[:], in1=negk[:], op=ALU.add)

                # gather prediction rows
                rows = sb.tile([P, 85], F32, tag="rows")
                nc.gpsimd.indirect_dma_start(
                    out=rows[:], out_offset=None, in_=pred_ap[:, :],
                    in_offset=IndirectOffsetOnAxis(ap=idx[:, :1], axis=0))

                # cls: bce_sum = sum_c(x_c + s_c) - x_{c*},  s_c = ln(1+e^-x_c)
                ecls = sb.tile([P, C], F32, tag="ecls")
                nc.scalar.activation(out=ecls[:], in_=rows[:, 5:85], func=AF.Exp, scale=-1.0)
                nc.vector.tensor_scalar_add(ecls[:], ecls[:], 1.0)
                scls = sb.tile([P, C], F32, tag="scls")
                ssum = sb.tile([P, 1], F32, tag="ssum")
                nc.scalar.activation(out=scls[:], in_=ecls[:], func=AF.Ln, accum_out=ssum[:])
                xsum = sb.tile([P, 1], F32, tag="xsum")
                nc.vector.reduce_sum(out=xsum[:], in_=rows[:, 5:85], axis=AX.X)
                oh = sb.tile([P, C], F32, tag="oh")
                nc.vector.tensor_tensor(out=oh[:], in0=iotaf[:P, :],
                                        in1=tt2[:, o:o + 1].to_broadcast([P, C]), op=ALU.is_equal)
                ohx = sb.tile([P, C], F32, tag="ohx")
                nc.vector.tensor_tensor(out=ohx[:], in0=oh[:], in1=rows[:, 5:85], op=ALU.mult)
                xstar = sb.tile([P, 1], F32, tag="xstar")
                nc.vector.reduce_sum(out=xstar[:], in_=ohx[:], axis=AX.X)
                pcls = sb.tile([P, 1], F32, tag="pcls")
                nc.vector.tensor_tensor(out=pcls[:], in0=ssum[:], in1=xsum[:], op=ALU.add)
                nc.vector.tensor_tensor(out=pcls[:], in0=pcls[:], in1=xstar[:], op=ALU.subtract)
                nc.vector.tensor_scalar_mul(pcls[:], pcls[:], 1.0 / C)

                # xy: sigma(x) = exp(-ln(1+e^-x))
                exy = sb.tile([P, 2], F32, tag="exy")
                nc.scalar.activation(out=exy[:], in_=rows[:, 0:2], func=AF.Exp, scale=-1.0)
                nc.vector.tensor_scalar_add(exy[:], exy[:], 1.0)
                sxy = sb.tile([P, 2], F32, tag="sxy")
                nc.scalar.activation(out=sxy[:], in_=exy[:], func=AF.Ln)
                sigxy = sb.tile([P, 2], F32, tag="sigxy")
                nc.scalar.activation(out=sigxy[:], in_=sxy[:], func=AF.Exp, scale=-1.0)
                txy = sb.tile([P, 2], F32, tag="txy")
                nc.vector.tensor_tensor(out=txy[:, 0:1], in0=xW[:], in1=gx[:], op=ALU.subtract)
                nc.vector.tensor_tensor(out=txy[:, 1:2], in0=yH[:], in1=gy[:], op=ALU.subtract)
                dxy = sb.tile([P, 2], F32, tag="dxy")
                nc.vector.tensor_tensor(out=dxy[:], in0=sigxy[:], in1=txy[:], op=ALU.subtract)
                nc.vector.tensor_tensor(out=dxy[:], in0=dxy[:], in1=dxy[:], op=ALU.mult)
                pxy = sb.tile([P, 1], F32, tag="pxy")
                nc.vector.reduce_sum(out=pxy[:], in_=dxy[:], axis=AX.X)
                nc.vector.tensor_scalar_mul(pxy[:], pxy[:], 0.5)

                # wh
                pwh_t = sb.tile([P, 2], F32, tag="pwh")
                nc.scalar.activation(out=pwh_t[:], in_=rows[:, 2:4], func=AF.Exp)
                twh = sb.tile([P, 2], F32, tag="twh")
                nc.vector.tensor_scalar_mul(twh[:, 0:1], tt2[:, o + 3:o + 4], float(W))
                nc.vector.tensor_scalar_mul(twh[:, 1:2], tt2[:, o + 4:o + 5], float(H))
                dwh = sb.tile([P, 2], F32, tag="dwh")
                nc.vector.tensor_tensor(out=dwh[:], in0=pwh_t[:], in1=twh[:], op=ALU.subtract)
                nc.vector.tensor_tensor(out=dwh[:], in0=dwh[:], in1=dwh[:], op=ALU.mult)
                pwh = sb.tile([P, 1], F32, tag="pwh1")
                nc.vector.reduce_sum(out=pwh[:], in_=dwh[:], axis=AX.X)
                nc.vector.tensor_scalar_mul(pwh[:], pwh[:], 0.5)

                # dedup: first-occurrence weight w (for obj_mask scatter-max)
                keyT_ps = ps.tile([P, P], F32, space="PSUM", tag="keyT_ps")
                nc.tensor.transpose(out=keyT_ps[:], in_=key[:].to_broadcast([P, P]),
                                    identity=ident[:P, :P])
                keyT = sb.tile([P, P], F32, tag="keyT")
                nc.vector.tensor_copy(out=keyT[:], in_=keyT_ps[:])
                eq = sb.tile([P, P], F32, tag="eq")
                nc.vector.tensor_tensor(out=eq[:], in0=key[:].to_broadcast([P, P]),
                                        in1=keyT[:], op=ALU.is_equal)
                nc.vector.tensor_tensor(out=eq[:], in0=eq[:], in1=tri[:P, :P], op=ALU.mult)
                dup = sb.tile([P, 1], F32, tag="dup")
                nc.vector.reduce_max(out=dup[:], in_=eq[:], axis=AX.X)
                wfo = sb.tile([P, 1], F32, tag="wfo")
                nc.vector.tensor_scalar(out=wfo[:], in0=dup[:], scalar1=-1.0, scalar2=1.0,
                                        op0=ALU.mult, op1=ALU.add)
                nc.vector.tensor_tensor(out=wfo[:], in0=wfo[:], in1=vf[:], op=ALU.mult)

                # stats columns: vf*pxy, vf*pwh, vf*pcls, vf, wfo*x4 (host negates)
                nc.vector.tensor_tensor(out=stats2[:, o + 0:o + 1], in0=pxy[:], in1=vf[:], op=ALU.mult)
                nc.vector.tensor_tensor(out=stats2[:, o + 1:o + 2], in0=pwh[:], in1=vf[:], op=ALU.mult)
                nc.vector.tensor_tensor(out=stats2[:, o + 2:o + 3], in0=pcls[:], in1=vf[:], op=ALU.mult)
                nc.vector.tensor_copy(out=stats2[:, o + 3:o + 4], in_=vf[:])
                nc.vector.tensor_tensor(out=stats2[:, o + 4:o + 5], in0=rows[:, 4:5], in1=wfo[:], op=ALU.mult)

            # stats out on gpsimd: fully hidden under the conf stream
            nc.gpsimd.dma_start(out=stats_d.ap()[:, :], in_=stats2[:])

            # ---- conf compute: softplus(x) = ln(1+e^x) summed per partition
            for k in range(NCH):
                ex = cp.tile([128, CW], F32, tag="conf_ex")
                nc.scalar.activation(out=ex[:], in_=conf_tl[k][:], func=AF.Exp)
                nc.vector.tensor_scalar_add(ex[:], ex[:], 1.0)
                lnt = cp.tile([128, CW], F32, tag="conf_ln")
                nc.scalar.activation(out=lnt[:], in_=ex[:], func=AF.Ln,
                                     accum_out=accT[:, k:k + 1])

            # acc out on scalar: follows ln7 in program order on the same
            # engine (no cross-engine waits) and uses the empty ACT HWDGE ring
            nc.scalar.dma_start(out=acc_d.ap()[:, :], in_=accT[:])
    if split:
        _split_multi_waits(nc)
    return nc


_NC_CACHE = None


def _get_nc():
    global _NC_CACHE
    if _NC_CACHE is None:
        _NC_CACHE = build_nc()
    return _NC_CACHE


def make_in_maps(predictions, targets):
    preds = np.ascontiguousarray(np.asarray(predictions, dtype=np.float32)).reshape(NCORES, ROWS, 85)
    tgts = np.ascontiguousarray(np.asarray(targets, dtype=np.float32)).reshape(NCORES, NT, 5)
    return [{"predictions": preds[c], "targets": tgts[c]} for c in range(NCORES)]


def combine_partials(results):
    """results: list of 8 dicts with 'stats' [100,10] and 'acc' [128,NCH]
    -> (total, loss_xy, loss_wh, loss_conf, loss_cls)"""
    st = np.sum([np.asarray(r["stats"], dtype=np.float64) for r in results], axis=(0, 1))
    sp_total = float(np.sum([np.asarray(r["acc"], dtype=np.float64) for r in results]))
    xy = st[0] + st[5]
    wh = st[1] + st[6]
    cls_ = st[2] + st[7]
    nt = st[3] + st[8]
    x4 = st[4] + st[9]
    denom = np.float32(max(float(nt), 1.0))
    loss_xy = np.float32(np.float32(xy) / denom)
    loss_wh = np.float32(np.float32(wh) / denom)
    loss_cls = np.float32(np.float32(cls_) / denom)
    loss_conf = np.float32((np.float32(sp_total) - np.float32(x4)) / np.float32(B * HWC))
    total = np.float32(5.0 * loss_xy + 5.0 * loss_wh + loss_conf + loss_cls)
    return total, loss_xy, loss_wh, loss_conf, loss_cls


def kernel(predictions, targets, H=None, W=None):
    from concourse.bass_utils import run_bass_kernel_spmd

    nc = _get_nc()
    in_maps = make_in_maps(predictions, targets)
    res = run_bass_kernel_spmd(nc, in_maps, core_ids=list(range(NCORES)))
    return combine_partials([res.results[c] for c in range(NCORES)])
